# revision 16
# baseline (speedup 1.0000x reference)
"""Distributed 2-layer GCN (PyG GCNConv semantics) on 8 Trainium2 NeuronCores.

Strategy: nodes sharded across 8 cores (12500 each); edges bucketed by
(dst-core, dst-tile-group, src-segment) via host-side 1D partitioning.
Three SPMD launches:

  1. transform1:  ht1 = dinv * (x @ W1)   (host-pretransposed bf16 x -> no
     PE transposes; single partition-major bf16 output table)
  2. agg1+xform2: gather ht1b[src] rows per edge (4 SWDGE queues), segmented
     one-hot matmul scatter-add per dst tile, self-loop added in PSUM via an
     identity matmul, relu, ht2 = dinv*(h1 @ W2); f32 partition-major out.
  3. agg2+logsoftmax over f32 ht2 table (64-feat rows = 256B gathers).

Perf notes vs the earlier version (2.15ms -> target < 1.1ms):
  - slot padding trimmed 25%->~3%: edges packed contiguously per
    (group, seg) bucket with *group-level* dst encodings (dl in [0,TG*128)),
    so matmul block boundaries no longer have to align with dst tiles.
    Boundary blocks simply run one matmul per touched tile; the group-level
    dl encoding makes the one-hot rows of other tiles zero automatically.
  - per-core runtime descriptor counts: num_idxs_reg is value_load'ed from
    a per-core counts table, so DMA descriptors track the real per-core
    edge count while the instruction stream stays identical across cores.
    (The first G-pool-depth groups gather their full capacity to flush
    stale SBUF so untrimmed pad blocks never feed NaNs to the PE.)
  - all regular DMA is big-packet: tables/own/outputs use partition-major
    [128, NT*F] layouts (the host undoes the permutation for free).
  - L2 self term rides the PSUM accumulator (identity-stationary matmul)
    instead of two DVE passes.
"""

import os
import sys
import types

for _p in ("/opt/trn_rl_repo", "/root/.axon_site/_ro/trn_rl_repo", "/root/.axon_site"):
    if os.path.isdir(_p) and _p not in sys.path:
        sys.path.insert(0, _p)

import numpy as np
import ml_dtypes

from concourse import bass, bacc, tile
from concourse.bass_utils import run_bass_kernel_spmd

mybir = bass.mybir
DT = bass.mybir.dt
ALU = mybir.AluOpType
ACTF = mybir.ActivationFunctionType
BF16 = ml_dtypes.bfloat16

# Runtime (register-sourced) gather counts are unusable in this runtime:
# reg_load from SBUF/DRAM halts the engine (NRT_EXEC_UNIT_UNRECOVERABLE).
RUNTIME_COUNTS = os.environ.get("GCN_RUNTIME_COUNTS", "0") == "1"
SINGLE_PACKET = os.environ.get("GCN_SINGLE_PACKET", "0") == "1"

# ----------------------------------------------------------------------------
# Configuration
# ----------------------------------------------------------------------------


class Cfg:
    def __init__(self, N=100000, E=1600000, F0=256, F1=128, F2=64,
                 NCORES=8, SEG=4, TG=4, GBUFS=3):
        self.N = N
        self.E = E
        self.F0 = F0
        self.F1 = F1
        self.F2 = F2
        self.NCORES = NCORES
        self.NPC = N // NCORES            # nodes per core
        self.NT = -(-self.NPC // 128)     # dst tiles per core
        self.LAST_ROWS = self.NPC - (self.NT - 1) * 128
        self.NTP = self.NT * 128          # padded rows per core
        self.NGL = NCORES * self.NTP      # global padded table rows
        self.SEG = SEG
        assert self.NGL % SEG == 0
        self.SEGSZ = self.NGL // SEG
        assert self.SEGSZ <= 32767
        self.TG = TG                      # dst tiles per gather group
        self.NG = -(-self.NT // TG)
        self.groups = [list(range(g * TG, min((g + 1) * TG, self.NT)))
                       for g in range(self.NG)]
        self.GBUFS = GBUFS                # gather pool depth == warm groups
        self.KB = F0 // 128               # k blocks for transform 1
        # transform-1 input chunking (tiles per chunk) for DMA/compute overlap
        self.CT = 14 if self.NT % 14 == 0 else self.NT
        self.NCH = self.NT // self.CT


class Meta:
    """Edge partitioning metadata; identical across cores (static program)."""
    pass


def preprocess(cfg, edge_index):
    """1D graph partitioning of the edge list. Pure integer index work."""
    src = np.asarray(edge_index[0], dtype=np.int64)
    dst = np.asarray(edge_index[1], dtype=np.int64)

    cnt = np.bincount(dst, minlength=cfg.N).astype(np.int64)

    core = dst // cfg.NPC
    within = dst % cfg.NPC
    tile_id = within // 128
    dloc = within % 128
    g_id = tile_id // cfg.TG
    t_in_g = tile_id % cfg.TG
    gdl = t_in_g * 128 + dloc             # group-level dst encoding

    # node permutation: node (c, t, p) -> table row c*NTP + p*NT + t
    sc = src // cfg.NPC
    sw = src % cfg.NPC
    st = sw // 128
    sp = sw % 128
    prow = sc * cfg.NTP + sp * cfg.NT + st
    seg = prow // cfg.SEGSZ
    sloc = prow % cfg.SEGSZ

    # bucket (core, group, seg); within bucket sort by (tile, sloc)
    bucket = (core * cfg.NG + g_id) * cfg.SEG + seg
    skey = (bucket * cfg.TG + t_in_g) * np.int64(cfg.SEGSZ) + sloc
    order = np.argsort(skey, kind="stable")
    sloc_s = sloc[order].astype(np.int16)
    gdl_s = gdl[order].astype(np.float32)
    tig_s = t_in_g[order].astype(np.int64)
    bucket_s = bucket[order]

    nbuckets = cfg.NCORES * cfg.NG * cfg.SEG
    bc = np.bincount(bucket, minlength=nbuckets).reshape(
        cfg.NCORES, cfg.NG, cfg.SEG)
    bstart = np.zeros(nbuckets + 1, np.int64)
    np.cumsum(bc.reshape(-1), out=bstart[1:])
    # per (core, g, s, t) counts for block ranges
    bct = np.zeros((cfg.NCORES, cfg.NG, cfg.SEG, cfg.TG), np.int64)
    np.add.at(bct, (core, g_id, seg, t_in_g), 1)

    m = Meta()
    if RUNTIME_COUNTS:
        # uniform capacities so gather pool buffers keep one shape: warm
        # groups initialize every byte, runtime-trimmed gathers never expose
        # uninitialized SBUF to the PE.
        capv = -(-bc.max(axis=(0, 1)) // 128) * 128             # [SEG]
        cap = np.broadcast_to(capv, (cfg.NG, cfg.SEG)).astype(np.int64).copy()
    else:
        # static counts: every slot is gathered (pads idx=0), so caps can be
        # per-bucket minima = ceil(max-over-cores / 128).
        cap = (-(-bc.max(axis=0) // 128) * 128).astype(np.int64)
    m.cap = cap
    m.soff = np.zeros((cfg.NG, cfg.SEG), np.int64)  # slot offset within group
    m.goff = np.zeros(cfg.NG + 1, np.int64)          # group slot offset, global
    for g in range(cfg.NG):
        off = 0
        for s in range(cfg.SEG):
            m.soff[g, s] = off
            off += int(cap[g, s])
        m.goff[g + 1] = m.goff[g] + off
    m.capg = [int(m.goff[g + 1] - m.goff[g]) for g in range(cfg.NG)]
    m.tot = int(m.goff[cfg.NG])

    # per-core slot arrays. Pad slots: dl=-1 always (one-hot rows stay zero);
    # idx=-1 for runtime-trimmed groups (negative tail = not gathered, must
    # match the count register exactly), idx=0 for warm full-capacity groups
    # (gathered harmlessly so every pool-buffer byte gets initialized).
    idx_all = np.full((cfg.NCORES, m.tot), -1, np.int16)
    dl_all = np.full((cfg.NCORES, m.tot), -1.0, np.float32)
    for c in range(cfg.NCORES):
        for g in range(cfg.NG):
            for s in range(cfg.SEG):
                b = (c * cfg.NG + g) * cfg.SEG + s
                k = int(bc[c, g, s])
                o = int(m.goff[g]) + int(m.soff[g, s])
                if k:
                    e0 = int(bstart[b])
                    idx_all[c, o:o + k] = sloc_s[e0:e0 + k]
                    dl_all[c, o:o + k] = gdl_s[e0:e0 + k]
                if not RUNTIME_COUNTS or g < cfg.GBUFS:
                    idx_all[c, o + k:o + int(cap[g, s])] = 0

    # block ranges per (g, t): union over cores, per segment; block index is
    # group-relative (matches the per-group dl tile).
    m.ranges = {}
    for g in range(cfg.NG):
        for j, t in enumerate(cfg.groups[g]):
            lst = []
            for s in range(cfg.SEG):
                lo, hi = None, None
                for c in range(cfg.NCORES):
                    k = int(bct[c, g, s, j])
                    if k == 0:
                        continue
                    st_ = int(bct[c, g, s, :j].sum())
                    b0 = (int(m.soff[g, s]) + st_) // 128
                    b1 = -(-(int(m.soff[g, s]) + st_ + k) // 128)
                    lo = b0 if lo is None else min(lo, b0)
                    hi = b1 if hi is None else max(hi, b1)
                if lo is not None:
                    lst.append((s, lo, hi))
            m.ranges[(g, j)] = lst

    # device layouts: per-group 16-wrap idx and 128-wrap dl, concatenated
    idx_dev = np.zeros((cfg.NCORES, 128, m.tot // 16), np.int16)
    dl_dev = np.zeros((cfg.NCORES, 128, m.tot // 128), np.float32)
    for g in range(cfg.NG):
        o, cg = int(m.goff[g]), m.capg[g]
        i16 = idx_all[:, o:o + cg].reshape(cfg.NCORES, cg // 16, 16)
        idx_dev[:, :, o // 16:(o + cg) // 16] = np.tile(
            i16.transpose(0, 2, 1), (1, 8, 1))
        d128 = dl_all[:, o:o + cg].reshape(cfg.NCORES, cg // 128, 128)
        dl_dev[:, :, o // 128:(o + cg) // 128] = d128.transpose(0, 2, 1)
    m.idx_dev = np.ascontiguousarray(idx_dev)
    m.dl_dev = np.ascontiguousarray(dl_dev)

    # runtime gather counts: real per-core count, except warm groups (full)
    cnts = np.zeros((cfg.NCORES, cfg.NG * cfg.SEG), np.int32)
    for c in range(cfg.NCORES):
        for g in range(cfg.NG):
            for s in range(cfg.SEG):
                k = int(bc[c, g, s])
                cnts[c, g * cfg.SEG + s] = int(cap[g, s]) if g < cfg.GBUFS else k
    m.cnts_dev = np.ascontiguousarray(
        np.broadcast_to(cnts[:, None, :], (cfg.NCORES, 128, cfg.NG * cfg.SEG)))

    # degree counts per core as f32 [128, NT] (node t*128+p <-> [p, t])
    pad = cfg.NTP - cfg.NPC
    cnt_dev = np.zeros((cfg.NCORES, 128, cfg.NT), np.float32)
    for c in range(cfg.NCORES):
        cc = cnt[c * cfg.NPC:(c + 1) * cfg.NPC]
        cc = np.concatenate([cc, np.zeros(pad, np.int64)])
        cnt_dev[c] = cc.reshape(cfg.NT, 128).T.astype(np.float32)
    m.cnt_dev = cnt_dev
    return m


# ----------------------------------------------------------------------------
# Program builders
# ----------------------------------------------------------------------------


def _dinv_tiles(nc, pool, cnt_in, cfg):
    """dinv = 1/sqrt(cnt + 1) as an SBUF [128, NT] f32 tile."""
    cnt_sb = pool.tile([128, cfg.NT], DT.float32, tag="cnt")
    nc.sync.dma_start(out=cnt_sb[:], in_=cnt_in[:])
    deg = pool.tile([128, cfg.NT], DT.float32, tag="deg")
    nc.vector.tensor_scalar_add(deg[:], cnt_sb[:], 1.0)
    sq = pool.tile([128, cfg.NT], DT.float32, tag="sq")
    nc.scalar.sqrt(sq[:], deg[:])
    dinv = pool.tile([128, cfg.NT], DT.float32, tag="dinv")
    nc.vector.reciprocal(dinv[:], sq[:])
    return dinv


def build_transform1(cfg):
    """ht1b = bf16(dinv * (x @ W1)), partition-major [128, NT*F1] output.

    x arrives host-transposed/bf16 as [128, NCH, KB, CT*128] (k-partition,
    chunk, k-block, node-col); W1 bf16 [128, KB, F1]."""
    nc = bacc.Bacc(None, target_bir_lowering=False)
    xT_in = nc.declare_dram_parameter(
        "xT", [128, cfg.NCH, cfg.KB, cfg.CT * 128], DT.bfloat16, isOutput=False)
    w1_in = nc.declare_dram_parameter("w1b", [128, cfg.KB, cfg.F1],
                                      DT.bfloat16, isOutput=False)
    cnt_in = nc.declare_dram_parameter("cnt", [128, cfg.NT], DT.float32,
                                       isOutput=False)
    out_t = nc.declare_dram_parameter("ht1b", [128, cfg.NT * cfg.F1],
                                      DT.bfloat16, isOutput=True)

    with tile.TileContext(nc) as tc:
        with tc.tile_pool(name="const", bufs=1) as cpool, \
             tc.tile_pool(name="xin", bufs=2) as xpool, \
             tc.tile_pool(name="psum", bufs=4, space="PSUM") as ppool:
            dinv = _dinv_tiles(nc, cpool, cnt_in, cfg)
            w1sb = cpool.tile([128, cfg.KB, cfg.F1], DT.bfloat16, tag="w1")
            nc.sync.dma_start(out=w1sb[:], in_=w1_in[:])
            outsb = cpool.tile([128, cfg.NT * cfg.F1], DT.bfloat16, tag="out")
            # pad partitions of the last tile are never computed; zero them so
            # the table/own rows they become stay finite.
            nc.vector.memset(outsb[:], 0.0)

            CW = cfg.CT * cfg.F1
            for ch in range(cfg.NCH):
                xt = xpool.tile([128, cfg.KB, cfg.CT * 128], DT.bfloat16,
                                tag="xt")
                nc.sync.dma_start(out=xt[:], in_=xT_in[:, ch, :, :])
                for lt in range(cfg.CT):
                    t = ch * cfg.CT + lt
                    rows = cfg.LAST_ROWS if t == cfg.NT - 1 else 128
                    hp = ppool.tile([128, cfg.F1], DT.float32, tag="hp")
                    for kb in range(cfg.KB):
                        nc.tensor.matmul(
                            hp[:rows, :],
                            xt[:, kb, lt * 128:lt * 128 + rows],
                            w1sb[:, kb, :],
                            start=(kb == 0), stop=(kb == cfg.KB - 1))
                    nc.vector.tensor_scalar_mul(
                        outsb[:rows, t * cfg.F1:(t + 1) * cfg.F1],
                        hp[:rows, :], dinv[:rows, t:t + 1])
                nc.scalar.dma_start(out=out_t[:, ch * CW:(ch + 1) * CW],
                                    in_=outsb[:, ch * CW:(ch + 1) * CW])
    nc.compile()
    return nc


def build_agg(cfg, meta, layer):
    """layer 1: aggregate ht1b -> h1 -> ht2 = dinv*(h1 @ W2); f32 out.
       layer 2: aggregate ht2 -> +b2 -> log_softmax; f32 out.

    Per group: SEG gathers (one per SWDGE queue) with runtime per-core
    descriptor counts; per dst tile a one-hot stationary built from the
    group-level dl encoding; self term accumulated into PSUM via an
    identity matmul; wide DVE eviction."""
    if layer == 1:
        FIN, gdt, sdt = cfg.F1, DT.bfloat16, DT.bfloat16
    else:
        FIN, gdt, sdt = cfg.F2, DT.float32, DT.float32
    FOUT = cfg.F2
    nc = bacc.Bacc(None, target_bir_lowering=False, num_swdge_queues=4)
    tab_in = [nc.declare_dram_parameter(f"tab{si}", [cfg.SEGSZ, FIN], gdt,
                                        isOutput=False)
              for si in range(cfg.SEG)]
    own_in = nc.declare_dram_parameter("own", [128, cfg.NT * FIN], gdt,
                                       isOutput=False)
    cnt_in = nc.declare_dram_parameter("cnt", [128, cfg.NT], DT.float32,
                                       isOutput=False)
    idx_in = nc.declare_dram_parameter("idx", [128, meta.tot // 16], DT.int16,
                                       isOutput=False)
    dl_in = nc.declare_dram_parameter("dl", [128, meta.tot // 128], DT.float32,
                                      isOutput=False)
    cnts_in = None
    if RUNTIME_COUNTS:
        cnts_in = nc.declare_dram_parameter("cnts", [128, cfg.NG * cfg.SEG],
                                            DT.int32, isOutput=False)
    iota_in = nc.declare_dram_parameter("iota", [128, cfg.TG * 128],
                                        DT.float32, isOutput=False)
    b_in = nc.declare_dram_parameter("bvec", [128, FIN], DT.float32,
                                     isOutput=False)
    ident_in = nc.declare_dram_parameter("ident", [128, 128], gdt,
                                         isOutput=False)
    if layer == 1:
        w2_in = nc.declare_dram_parameter("w2b", [128, cfg.F2], DT.bfloat16,
                                          isOutput=False)
        out_t = nc.declare_dram_parameter("ht2", [128, cfg.NT * cfg.F2],
                                          DT.float32, isOutput=True)
    else:
        out_t = nc.declare_dram_parameter("out", [128, cfg.NT * cfg.F2],
                                          DT.float32, isOutput=True)

    with tile.TileContext(nc) as tc:
        with tc.tile_pool(name="const", bufs=1) as cpool, \
             tc.tile_pool(name="idx", bufs=cfg.GBUFS) as idxpool, \
             tc.tile_pool(name="dl", bufs=cfg.GBUFS) as dlpool, \
             tc.tile_pool(name="g0", bufs=cfg.GBUFS) as g0, \
             tc.tile_pool(name="g1", bufs=cfg.GBUFS) as g1, \
             tc.tile_pool(name="g2", bufs=cfg.GBUFS) as g2, \
             tc.tile_pool(name="g3", bufs=cfg.GBUFS) as g3, \
             tc.tile_pool(name="own", bufs=2) as ownpool, \
             tc.tile_pool(name="S", bufs=2) as spool, \
             tc.tile_pool(name="work", bufs=3) as wpool, \
             tc.tile_pool(name="pacc", bufs=2, space="PSUM") as pacc, \
             tc.tile_pool(name="pmisc", bufs=2, space="PSUM") as pmisc:
            gpools = [g0, g1, g2, g3][:cfg.SEG]
            dinv = _dinv_tiles(nc, cpool, cnt_in, cfg)
            bvec = cpool.tile([128, FIN], DT.float32, tag="bvec")
            nc.sync.dma_start(out=bvec[:], in_=b_in[:])
            ident = cpool.tile([128, 128], gdt, tag="ident")
            nc.sync.dma_start(out=ident[:], in_=ident_in[:])
            iota = cpool.tile([128, cfg.TG * 128], DT.float32, tag="iota")
            nc.sync.dma_start(out=iota[:], in_=iota_in[:])
            if RUNTIME_COUNTS:
                cnts_sb = cpool.tile([128, cfg.NG * cfg.SEG], DT.int32,
                                     tag="cnts")
                nc.sync.dma_start(out=cnts_sb[:], in_=cnts_in[:])
            if layer == 1:
                w2b = cpool.tile([128, cfg.F2], DT.bfloat16, tag="w2b")
                nc.sync.dma_start(out=w2b[:], in_=w2_in[:])
                identb = ident  # bf16 already
            for g in range(cfg.NG):
                tiles = cfg.groups[g]
                ntg = len(tiles)
                t0 = tiles[0]
                go, cg = int(meta.goff[g]), meta.capg[g]

                idxg = idxpool.tile([128, cg // 16], DT.int16, tag="idxg")
                nc.sync.dma_start(out=idxg[:],
                                  in_=idx_in[:, go // 16:(go + cg) // 16])
                dlg = dlpool.tile([128, cg // 128], DT.float32, tag="dlg")
                nc.sync.dma_start(out=dlg[:],
                                  in_=dl_in[:, go // 128:(go + cg) // 128])

                Gt = {}
                for s in range(cfg.SEG):
                    csz = int(meta.cap[g, s])
                    if csz == 0:
                        continue
                    so = int(meta.soff[g, s])
                    if RUNTIME_COUNTS:
                        reg = nc.gpsimd.value_load(
                            cnts_sb[0:1, g * cfg.SEG + s:g * cfg.SEG + s + 1],
                            min_val=0, max_val=csz)
                    else:
                        reg = csz
                    Gs = gpools[s].tile([128, csz // 128, FIN], gdt,
                                        tag=f"G{s}")
                    nc.gpsimd.dma_gather(
                        out_ap=Gs[:],
                        in_ap=tab_in[s][:, :],
                        idxs_ap=idxg[:, so // 16:(so + csz) // 16],
                        num_idxs=csz,
                        num_idxs_reg=reg,
                        elem_size=FIN,
                        single_packet=SINGLE_PACKET,
                        queue_num=s,
                    )
                    Gt[s] = Gs

                ownw = ownpool.tile([128, ntg, FIN], gdt, tag="own")
                nc.sync.dma_start(
                    out=ownw[:],
                    in_=own_in[:, t0 * FIN:(t0 + ntg) * FIN]
                    .rearrange("p (a f) -> p a f", f=FIN))

                acc_g = pacc.tile([128, ntg, FIN], DT.float32, tag="acc")
                for j, t in enumerate(tiles):
                    rng = meta.ranges.get((g, j), [])
                    nbt = sum(b1 - b0 for _, b0, b1 in rng)
                    # self term first (start), aggregation blocks after
                    nc.tensor.matmul(acc_g[:, j, :], ident[:], ownw[:, j, :],
                                     start=True, stop=(nbt == 0))
                    if nbt == 0:
                        continue
                    St = spool.tile([128, nbt, 128], sdt, tag="St")
                    ci = 0
                    for s, b0, b1 in rng:
                        nb = b1 - b0
                        nc.vector.tensor_tensor(
                            St[:, ci:ci + nb, :],
                            iota[:, j * 128:(j + 1) * 128]
                            .unsqueeze(1).broadcast_to((128, nb, 128)),
                            dlg[:, b0:b1].unsqueeze(2)
                            .broadcast_to((128, nb, 128)),
                            op=ALU.is_equal)
                        ci += nb
                    ci = 0
                    for s, b0, b1 in rng:
                        base = int(meta.soff[g, s]) // 128
                        for k in range(b0, b1):
                            nc.tensor.matmul(
                                acc_g[:, j, :], St[:, ci, :],
                                Gt[s][:, k - base, :],
                                start=False,
                                stop=(ci == nbt - 1))
                            ci += 1

                dinv_bc = dinv[:, t0:t0 + ntg].unsqueeze(2).broadcast_to(
                    (128, ntg, FIN))
                bvec_bc = bvec[:].unsqueeze(1).broadcast_to((128, ntg, FIN))
                zw = wpool.tile([128, ntg, FIN], DT.float32, tag="zw")
                nc.vector.tensor_tensor(zw[:], acc_g[:], dinv_bc, op=ALU.mult)
                nc.vector.tensor_tensor(zw[:], zw[:], bvec_bc, op=ALU.add)

                if layer == 1:
                    h1b = wpool.tile([128, ntg, cfg.F1], DT.bfloat16,
                                     tag="h1b")
                    nc.scalar.activation(h1b[:], zw[:], ACTF.Relu)
                    t2p = pmisc.tile([128, ntg, cfg.F2], DT.float32,
                                     tag="t2p")
                    for j in range(ntg):
                        hTp = pmisc.tile([128, 128], DT.bfloat16, tag="hTp")
                        nc.tensor.transpose(hTp[:], h1b[:, j, :], identb[:])
                        hTb = wpool.tile([128, 128], DT.bfloat16, tag="hTb")
                        nc.scalar.activation(hTb[:], hTp[:], ACTF.Copy)
                        nc.tensor.matmul(t2p[:, j, :], hTb[:], w2b[:],
                                         start=True, stop=True)
                    dinv_bc2 = dinv[:, t0:t0 + ntg].unsqueeze(2).broadcast_to(
                        (128, ntg, cfg.F2))
                    ht2w = wpool.tile([128, ntg, cfg.F2], DT.float32,
                                      tag="ht2w")
                    nc.vector.tensor_tensor(ht2w[:], t2p[:], dinv_bc2,
                                            op=ALU.mult)
                    nc.scalar.dma_start(
                        out=out_t[:, t0 * cfg.F2:(t0 + ntg) * cfg.F2],
                        in_=ht2w[:].rearrange("p a f -> p (a f)"))
                else:
                    mxw = wpool.tile([128, ntg], DT.float32, tag="mxw")
                    nc.vector.tensor_reduce(mxw[:], zw[:],
                                            axis=mybir.AxisListType.X,
                                            op=ALU.max, negate=True)
                    ssumw = wpool.tile([128, ntg], DT.float32, tag="ssumw")
                    for j in range(ntg):
                        e = wpool.tile([128, cfg.F2], DT.float32, tag="e")
                        nc.scalar.activation(e[:], zw[:, j, :], ACTF.Exp,
                                             bias=mxw[:, j:j + 1],
                                             accum_out=ssumw[:, j:j + 1])
                    lsew = wpool.tile([128, ntg], DT.float32, tag="lsew")
                    nc.scalar.activation(lsew[:], ssumw[:], ACTF.Ln)
                    nc.vector.tensor_tensor(
                        zw[:], zw[:],
                        mxw[:].unsqueeze(2).broadcast_to((128, ntg, cfg.F2)),
                        op=ALU.add)
                    nc.vector.tensor_tensor(
                        zw[:], zw[:],
                        lsew[:].unsqueeze(2).broadcast_to((128, ntg, cfg.F2)),
                        op=ALU.subtract)
                    nc.scalar.dma_start(
                        out=out_t[:, t0 * cfg.F2:(t0 + ntg) * cfg.F2],
                        in_=zw[:].rearrange("p a f -> p (a f)"))
    nc.compile()
    return nc


# ----------------------------------------------------------------------------
# Runner
# ----------------------------------------------------------------------------


def _install_ntff_hook():
    try:
        import antenv
        if "antenv.axon_hooks" not in sys.modules:
            from trn_agent_boot.trn_boot import _ntff_profile_via_ctypes
            hooks = types.ModuleType("antenv.axon_hooks")
            holder = {"hook": _ntff_profile_via_ctypes("/opt/axon/libaxon_pjrt.so")}
            hooks.get_axon_ntff_profile_hook = lambda: holder["hook"]
            hooks.set_axon_ntff_profile_hook = lambda h: holder.__setitem__("hook", h)
            sys.modules["antenv.axon_hooks"] = hooks
            antenv.axon_hooks = hooks
    except Exception:
        pass


_CACHE = {}
LAST_EXEC_NS = []


def _get_programs(cfg, meta, key):
    if key not in _CACHE:
        _CACHE[key] = (build_transform1(cfg),
                       build_agg(cfg, meta, 1),
                       build_agg(cfg, meta, 2))
    return _CACHE[key]


def _marshal_xT(cfg, x):
    """x [N, F0] f32 -> per-core [128, NCH, KB, CT*128] bf16 (k-major)."""
    out = np.zeros((cfg.NCORES, 128, cfg.NCH, cfg.KB, cfg.CT * 128), BF16)
    for c in range(cfg.NCORES):
        xs = x[c * cfg.NPC:(c + 1) * cfg.NPC]          # [NPC, F0]
        xp = np.zeros((cfg.NTP, cfg.F0), np.float32)
        xp[:cfg.NPC] = xs
        # [NT*128, KB, 128] -> [128(k), NCH, KB, CT*128(node)]
        xk = xp.reshape(cfg.NTP, cfg.KB, 128)
        xk = xk.transpose(2, 1, 0).reshape(128, cfg.KB, cfg.NCH, cfg.CT * 128)
        out[c] = xk.transpose(0, 2, 1, 3).astype(BF16)
    return out


def kernel(x, edge_index, W1, b1, W2, b2):
    cfg = Cfg()
    x = np.asarray(x, dtype=np.float32)
    edge_index = np.asarray(edge_index)
    W1 = np.asarray(W1, dtype=np.float32)
    b1 = np.asarray(b1, dtype=np.float32)
    W2 = np.asarray(W2, dtype=np.float32)
    b2 = np.asarray(b2, dtype=np.float32)

    trace = os.environ.get("GCN_TRACE", "0") == "1"
    if trace:
        _install_ntff_hook()

    meta = preprocess(cfg, edge_index)
    key = hash(edge_index.tobytes())
    p1, p2, p3 = _get_programs(cfg, meta, key)

    xT = _marshal_xT(cfg, x)
    w1b = np.ascontiguousarray(
        W1.reshape(cfg.KB, 128, cfg.F1).transpose(1, 0, 2)).astype(BF16)
    w2b = W2.astype(BF16)
    identb_v = np.eye(128, dtype=np.float32).astype(BF16)
    identf_v = np.eye(128, dtype=np.float32)
    iota_v = np.broadcast_to(
        np.arange(cfg.TG * 128, dtype=np.float32), (128, cfg.TG * 128)).copy()
    b1b = np.broadcast_to(b1, (128, cfg.F1)).copy()
    b2b = np.broadcast_to(b2, (128, cfg.F2)).copy()
    cores = list(range(cfg.NCORES))

    global LAST_EXEC_NS
    LAST_EXEC_NS = []

    # Launch 1: transform
    maps1 = [{"xT": xT[c], "w1b": w1b, "cnt": meta.cnt_dev[c]}
             for c in cores]
    r1 = run_bass_kernel_spmd(p1, maps1, cores, trace=trace)
    LAST_EXEC_NS.append(r1.exec_time_ns)
    ht1b = np.stack([r1.results[c]["ht1b"] for c in cores], axis=0)
    # global table: row c*NTP + p*NT + t <- ht1b[c][p, t*F1:+F1]
    tabg1 = ht1b.reshape(cfg.NCORES * 128 * cfg.NT, cfg.F1)
    segs1 = {f"tab{si}": np.ascontiguousarray(
        tabg1[si * cfg.SEGSZ:(si + 1) * cfg.SEGSZ])
        for si in range(cfg.SEG)}

    # Launch 2: layer-1 aggregation + transform-2
    maps2 = [{**segs1, "own": ht1b[c], "cnt": meta.cnt_dev[c],
              "idx": meta.idx_dev[c], "dl": meta.dl_dev[c],
              **({"cnts": meta.cnts_dev[c]} if RUNTIME_COUNTS else {}),
              "iota": iota_v, "bvec": b1b,
              "ident": identb_v, "w2b": w2b} for c in cores]
    r2 = run_bass_kernel_spmd(p2, maps2, cores, trace=trace)
    LAST_EXEC_NS.append(r2.exec_time_ns)
    ht2 = np.stack([r2.results[c]["ht2"] for c in cores], axis=0)
    tabg2 = ht2.reshape(cfg.NCORES * 128 * cfg.NT, cfg.F2)
    segs2 = {f"tab{si}": np.ascontiguousarray(
        tabg2[si * cfg.SEGSZ:(si + 1) * cfg.SEGSZ])
        for si in range(cfg.SEG)}

    # Launch 3: layer-2 aggregation + log_softmax
    maps3 = [{**segs2, "own": ht2[c], "cnt": meta.cnt_dev[c],
              "idx": meta.idx_dev[c], "dl": meta.dl_dev[c],
              **({"cnts": meta.cnts_dev[c]} if RUNTIME_COUNTS else {}),
              "iota": iota_v, "bvec": b2b,
              "ident": identf_v} for c in cores]
    r3 = run_bass_kernel_spmd(p3, maps3, cores, trace=trace)
    LAST_EXEC_NS.append(r3.exec_time_ns)

    out = np.empty((cfg.N, cfg.F2), np.float32)
    for c in cores:
        o = r3.results[c]["out"].reshape(128, cfg.NT, cfg.F2)
        out[c * cfg.NPC:(c + 1) * cfg.NPC] = (
            o.transpose(1, 0, 2).reshape(cfg.NTP, cfg.F2)[:cfg.NPC])
    return out


# revision 25
# speedup vs baseline: 1.2375x; 1.2375x over previous
"""Distributed 2-layer GCN (PyG GCNConv semantics) on 8 Trainium2 NeuronCores.

Strategy: nodes sharded across 8 cores (12500 each); edges bucketed by
(dst-core, dst-tile-group, src-segment) via host-side 1D partitioning.
Three SPMD launches:

  1. transform1:  ht1 = dinv * (x @ W1)   (host-pretransposed bf16 x -> no
     PE transposes; single partition-major bf16 output table)
  2. agg1+xform2: gather ht1b[src] rows per edge (4 SWDGE queues), segmented
     one-hot matmul scatter-add per dst tile, self-loop added in PSUM via an
     identity matmul, relu, ht2 = dinv*(h1 @ W2); f32 partition-major out.
  3. agg2+logsoftmax over f32 ht2 table (64-feat rows = 256B gathers).

Perf notes vs the earlier version (2.15ms -> target < 1.1ms):
  - slot padding trimmed 25%->~3%: edges packed contiguously per
    (group, seg) bucket with *group-level* dst encodings (dl in [0,TG*128)),
    so matmul block boundaries no longer have to align with dst tiles.
    Boundary blocks simply run one matmul per touched tile; the group-level
    dl encoding makes the one-hot rows of other tiles zero automatically.
  - per-core runtime descriptor counts: num_idxs_reg is value_load'ed from
    a per-core counts table, so DMA descriptors track the real per-core
    edge count while the instruction stream stays identical across cores.
    (The first G-pool-depth groups gather their full capacity to flush
    stale SBUF so untrimmed pad blocks never feed NaNs to the PE.)
  - all regular DMA is big-packet: tables/own/outputs use partition-major
    [128, NT*F] layouts (the host undoes the permutation for free).
  - L2 self term rides the PSUM accumulator (identity-stationary matmul)
    instead of two DVE passes.
"""

import os
import sys
import types

for _p in ("/opt/trn_rl_repo", "/root/.axon_site/_ro/trn_rl_repo", "/root/.axon_site"):
    if os.path.isdir(_p) and _p not in sys.path:
        sys.path.insert(0, _p)

import numpy as np
import ml_dtypes

from concourse import bass, bacc, tile
from concourse.bass_utils import run_bass_kernel_spmd

mybir = bass.mybir
DT = bass.mybir.dt
ALU = mybir.AluOpType
ACTF = mybir.ActivationFunctionType
BF16 = ml_dtypes.bfloat16

# Runtime (register-sourced) gather counts are unusable in this runtime:
# reg_load from SBUF/DRAM halts the engine (NRT_EXEC_UNIT_UNRECOVERABLE).
RUNTIME_COUNTS = os.environ.get("GCN_RUNTIME_COUNTS", "0") == "1"
SINGLE_PACKET = os.environ.get("GCN_SINGLE_PACKET", "0") == "1"

# ----------------------------------------------------------------------------
# Configuration
# ----------------------------------------------------------------------------


class Cfg:
    def __init__(self, N=100000, E=1600000, F0=256, F1=128, F2=64,
                 NCORES=8, SEG=4, TG=7, GBUFS=3):
        self.N = N
        self.E = E
        self.F0 = F0
        self.F1 = F1
        self.F2 = F2
        self.NCORES = NCORES
        self.NPC = N // NCORES            # nodes per core
        self.NT = -(-self.NPC // 128)     # dst tiles per core
        self.LAST_ROWS = self.NPC - (self.NT - 1) * 128
        self.NTP = self.NT * 128          # padded rows per core
        self.NGL = NCORES * self.NTP      # global padded table rows
        self.SEG = SEG
        assert self.NGL % SEG == 0
        self.SEGSZ = self.NGL // SEG
        assert self.SEGSZ <= 32767
        self.TG = TG                      # dst tiles per gather group
        self.NG = -(-self.NT // TG)
        self.groups = [list(range(g * TG, min((g + 1) * TG, self.NT)))
                       for g in range(self.NG)]
        self.GBUFS = GBUFS                # gather pool depth == warm groups
        self.KB = F0 // 128               # k blocks for transform 1
        # transform-1 input chunking (tiles per chunk) for DMA/compute overlap
        self.CT = 14 if self.NT % 14 == 0 else self.NT
        self.NCH = self.NT // self.CT


class Meta:
    """Edge partitioning metadata; identical across cores (static program)."""
    pass


def preprocess(cfg, edge_index):
    """1D graph partitioning of the edge list. Pure integer index work."""
    src = np.asarray(edge_index[0], dtype=np.int64)
    dst = np.asarray(edge_index[1], dtype=np.int64)

    cnt = np.bincount(dst, minlength=cfg.N).astype(np.int64)

    core = dst // cfg.NPC
    within = dst % cfg.NPC
    tile_id = within // 128
    dloc = within % 128
    g_id = tile_id // cfg.TG
    t_in_g = tile_id % cfg.TG
    gdl = t_in_g * 128 + dloc             # group-level dst encoding

    # node permutation: node (c, t, p) -> table row c*NTP + p*NT + t
    sc = src // cfg.NPC
    sw = src % cfg.NPC
    st = sw // 128
    sp = sw % 128
    prow = sc * cfg.NTP + sp * cfg.NT + st
    seg = prow // cfg.SEGSZ
    sloc = prow % cfg.SEGSZ

    # bucket (core, group, seg); within bucket sort by (tile, sloc)
    bucket = (core * cfg.NG + g_id) * cfg.SEG + seg
    skey = (bucket * cfg.TG + t_in_g) * np.int64(cfg.SEGSZ) + sloc
    order = np.argsort(skey, kind="stable")
    sloc_s = sloc[order].astype(np.int16)
    gdl_s = gdl[order].astype(np.float32)
    tig_s = t_in_g[order].astype(np.int64)
    bucket_s = bucket[order]

    nbuckets = cfg.NCORES * cfg.NG * cfg.SEG
    bc = np.bincount(bucket, minlength=nbuckets).reshape(
        cfg.NCORES, cfg.NG, cfg.SEG)
    bstart = np.zeros(nbuckets + 1, np.int64)
    np.cumsum(bc.reshape(-1), out=bstart[1:])
    # per (core, g, s, t) counts for block ranges
    bct = np.zeros((cfg.NCORES, cfg.NG, cfg.SEG, cfg.TG), np.int64)
    np.add.at(bct, (core, g_id, seg, t_in_g), 1)

    m = Meta()
    if RUNTIME_COUNTS:
        # uniform capacities so gather pool buffers keep one shape: warm
        # groups initialize every byte, runtime-trimmed gathers never expose
        # uninitialized SBUF to the PE.
        capv = -(-bc.max(axis=(0, 1)) // 128) * 128             # [SEG]
        cap = np.broadcast_to(capv, (cfg.NG, cfg.SEG)).astype(np.int64).copy()
    else:
        # static counts: every slot is gathered (pads idx=0), so caps can be
        # per-bucket minima = ceil(max-over-cores / 128).
        cap = (-(-bc.max(axis=0) // 128) * 128).astype(np.int64)
    m.cap = cap
    m.soff = np.zeros((cfg.NG, cfg.SEG), np.int64)  # slot offset within group
    m.goff = np.zeros(cfg.NG + 1, np.int64)          # group slot offset, global
    for g in range(cfg.NG):
        off = 0
        for s in range(cfg.SEG):
            m.soff[g, s] = off
            off += int(cap[g, s])
        m.goff[g + 1] = m.goff[g] + off
    m.capg = [int(m.goff[g + 1] - m.goff[g]) for g in range(cfg.NG)]
    m.tot = int(m.goff[cfg.NG])

    # per-core slot arrays. Pad slots: dl=-1 always (one-hot rows stay zero);
    # idx=-1 for runtime-trimmed groups (negative tail = not gathered, must
    # match the count register exactly), idx=0 for warm full-capacity groups
    # (gathered harmlessly so every pool-buffer byte gets initialized).
    idx_all = np.full((cfg.NCORES, m.tot), -1, np.int16)
    dl_all = np.full((cfg.NCORES, m.tot), -1.0, np.float32)
    for c in range(cfg.NCORES):
        for g in range(cfg.NG):
            for s in range(cfg.SEG):
                b = (c * cfg.NG + g) * cfg.SEG + s
                k = int(bc[c, g, s])
                o = int(m.goff[g]) + int(m.soff[g, s])
                if k:
                    e0 = int(bstart[b])
                    idx_all[c, o:o + k] = sloc_s[e0:e0 + k]
                    dl_all[c, o:o + k] = gdl_s[e0:e0 + k]
                if not RUNTIME_COUNTS or g < cfg.GBUFS:
                    idx_all[c, o + k:o + int(cap[g, s])] = 0

    # block ranges per (g, t): union over cores, per segment; block index is
    # group-relative (matches the per-group dl tile).
    m.ranges = {}
    for g in range(cfg.NG):
        for j, t in enumerate(cfg.groups[g]):
            lst = []
            for s in range(cfg.SEG):
                lo, hi = None, None
                for c in range(cfg.NCORES):
                    k = int(bct[c, g, s, j])
                    if k == 0:
                        continue
                    st_ = int(bct[c, g, s, :j].sum())
                    b0 = (int(m.soff[g, s]) + st_) // 128
                    b1 = -(-(int(m.soff[g, s]) + st_ + k) // 128)
                    lo = b0 if lo is None else min(lo, b0)
                    hi = b1 if hi is None else max(hi, b1)
                if lo is not None:
                    lst.append((s, lo, hi))
            m.ranges[(g, j)] = lst

    # device layouts: per-group 16-wrap idx and 128-wrap dl, concatenated
    idx_dev = np.zeros((cfg.NCORES, 128, m.tot // 16), np.int16)
    dl_dev = np.zeros((cfg.NCORES, 128, m.tot // 128), np.float32)
    for g in range(cfg.NG):
        o, cg = int(m.goff[g]), m.capg[g]
        i16 = idx_all[:, o:o + cg].reshape(cfg.NCORES, cg // 16, 16)
        idx_dev[:, :, o // 16:(o + cg) // 16] = np.tile(
            i16.transpose(0, 2, 1), (1, 8, 1))
        d128 = dl_all[:, o:o + cg].reshape(cfg.NCORES, cg // 128, 128)
        dl_dev[:, :, o // 128:(o + cg) // 128] = d128.transpose(0, 2, 1)
    m.idx_dev = np.ascontiguousarray(idx_dev)
    m.dl_dev = np.ascontiguousarray(dl_dev)

    # runtime gather counts: real per-core count, except warm groups (full)
    cnts = np.zeros((cfg.NCORES, cfg.NG * cfg.SEG), np.int32)
    for c in range(cfg.NCORES):
        for g in range(cfg.NG):
            for s in range(cfg.SEG):
                k = int(bc[c, g, s])
                cnts[c, g * cfg.SEG + s] = int(cap[g, s]) if g < cfg.GBUFS else k
    m.cnts_dev = np.ascontiguousarray(
        np.broadcast_to(cnts[:, None, :], (cfg.NCORES, 128, cfg.NG * cfg.SEG)))

    # degree counts per core as f32 [128, NT] (node t*128+p <-> [p, t])
    pad = cfg.NTP - cfg.NPC
    cnt_dev = np.zeros((cfg.NCORES, 128, cfg.NT), np.float32)
    for c in range(cfg.NCORES):
        cc = cnt[c * cfg.NPC:(c + 1) * cfg.NPC]
        cc = np.concatenate([cc, np.zeros(pad, np.int64)])
        cnt_dev[c] = cc.reshape(cfg.NT, 128).T.astype(np.float32)
    m.cnt_dev = cnt_dev
    return m


# ----------------------------------------------------------------------------
# Program builders
# ----------------------------------------------------------------------------


def _dinv_tiles(nc, pool, cnt_in, cfg):
    """dinv = 1/sqrt(cnt + 1) as an SBUF [128, NT] f32 tile."""
    cnt_sb = pool.tile([128, cfg.NT], DT.float32, tag="cnt")
    nc.scalar.dma_start(out=cnt_sb[:], in_=cnt_in[:])
    deg = pool.tile([128, cfg.NT], DT.float32, tag="deg")
    nc.vector.tensor_scalar_add(deg[:], cnt_sb[:], 1.0)
    sq = pool.tile([128, cfg.NT], DT.float32, tag="sq")
    nc.scalar.sqrt(sq[:], deg[:])
    dinv = pool.tile([128, cfg.NT], DT.float32, tag="dinv")
    nc.vector.reciprocal(dinv[:], sq[:])
    return dinv


def build_transform1(cfg):
    """ht1b = bf16(dinv * (x @ W1)), partition-major [128, NT*F1] output.

    x arrives host-transposed/bf16 as [128, NCH, KB, CT*128] (k-partition,
    chunk, k-block, node-col); W1 bf16 [128, KB, F1]."""
    nc = bacc.Bacc(None, target_bir_lowering=False)
    xT_in = nc.declare_dram_parameter(
        "xT", [128, cfg.NCH, cfg.KB, cfg.CT * 128], DT.bfloat16, isOutput=False)
    w1_in = nc.declare_dram_parameter("w1b", [128, cfg.KB, cfg.F1],
                                      DT.bfloat16, isOutput=False)
    cnt_in = nc.declare_dram_parameter("cnt", [128, cfg.NT], DT.float32,
                                       isOutput=False)
    out_t = nc.declare_dram_parameter("ht1b", [128, cfg.NT * cfg.F1],
                                      DT.bfloat16, isOutput=True)

    with tile.TileContext(nc) as tc:
        with tc.tile_pool(name="const", bufs=1) as cpool, \
             tc.tile_pool(name="xin", bufs=2) as xpool, \
             tc.tile_pool(name="psum", bufs=4, space="PSUM") as ppool:
            dinv = _dinv_tiles(nc, cpool, cnt_in, cfg)
            w1sb = cpool.tile([128, cfg.KB, cfg.F1], DT.bfloat16, tag="w1")
            nc.sync.dma_start(out=w1sb[:], in_=w1_in[:])
            outsb = cpool.tile([128, cfg.NT * cfg.F1], DT.bfloat16, tag="out")
            # pad partitions of the last tile are never computed; zero them so
            # the table/own rows they become stay finite.
            nc.vector.memset(outsb[:], 0.0)

            CW = cfg.CT * cfg.F1
            for ch in range(cfg.NCH):
                xt = xpool.tile([128, cfg.KB, cfg.CT * 128], DT.bfloat16,
                                tag="xt")
                nc.sync.dma_start(out=xt[:], in_=xT_in[:, ch, :, :])
                for lt in range(cfg.CT):
                    t = ch * cfg.CT + lt
                    rows = cfg.LAST_ROWS if t == cfg.NT - 1 else 128
                    hp = ppool.tile([128, cfg.F1], DT.float32, tag="hp")
                    for kb in range(cfg.KB):
                        nc.tensor.matmul(
                            hp[:rows, :],
                            xt[:, kb, lt * 128:lt * 128 + rows],
                            w1sb[:, kb, :],
                            start=(kb == 0), stop=(kb == cfg.KB - 1))
                    nc.vector.tensor_scalar_mul(
                        outsb[:rows, t * cfg.F1:(t + 1) * cfg.F1],
                        hp[:rows, :], dinv[:rows, t:t + 1])
                nc.scalar.dma_start(out=out_t[:, ch * CW:(ch + 1) * CW],
                                    in_=outsb[:, ch * CW:(ch + 1) * CW])
    nc.compile()
    return nc


def build_agg(cfg, meta, layer):
    """layer 1: aggregate ht1b -> h1 -> ht2 = dinv*(h1 @ W2); f32 out.
       layer 2: aggregate ht2 -> +b2 -> log_softmax; f32 out.

    Per group: SEG gathers (one per SWDGE queue) with runtime per-core
    descriptor counts; per dst tile a one-hot stationary built from the
    group-level dl encoding; self term accumulated into PSUM via an
    identity matmul; wide DVE eviction."""
    # FIN = aggregated feature width; FROW = table row width (256B bf16 rows:
    # layer-2 rows hold the 64 features duplicated). fp32 tables/matmuls are a
    # trap: fp32 stationary loads don't pipeline and the PE becomes the wall.
    FIN = cfg.F1 if layer == 1 else cfg.F2
    FROW = 128
    gdt = sdt = DT.bfloat16
    nc = bacc.Bacc(None, target_bir_lowering=False, num_swdge_queues=4)
    tab_in = [nc.declare_dram_parameter(f"tab{si}", [cfg.SEGSZ, FROW], gdt,
                                        isOutput=False)
              for si in range(cfg.SEG)]
    own_in = nc.declare_dram_parameter("own", [128, cfg.NT * FROW], gdt,
                                       isOutput=False)
    cnt_in = nc.declare_dram_parameter("cnt", [128, cfg.NT], DT.float32,
                                       isOutput=False)
    idx_in = nc.declare_dram_parameter("idx", [128, meta.tot // 16], DT.int16,
                                       isOutput=False)
    dl_in = nc.declare_dram_parameter("dl", [128, meta.tot // 128], DT.float32,
                                      isOutput=False)
    cnts_in = None
    if RUNTIME_COUNTS:
        cnts_in = nc.declare_dram_parameter("cnts", [128, cfg.NG * cfg.SEG],
                                            DT.int32, isOutput=False)
    iota_in = nc.declare_dram_parameter("iota", [128, cfg.TG * 128],
                                        DT.float32, isOutput=False)
    b_in = nc.declare_dram_parameter("bvec", [128, FIN], DT.float32,
                                     isOutput=False)
    ident_in = nc.declare_dram_parameter("ident", [128, 128], gdt,
                                         isOutput=False)
    if layer == 1:
        w2_in = nc.declare_dram_parameter("w2b", [128, cfg.F2], DT.bfloat16,
                                          isOutput=False)
        out_t = nc.declare_dram_parameter("ht2b", [128, cfg.NT * FROW],
                                          DT.bfloat16, isOutput=True)
    else:
        out_t = nc.declare_dram_parameter("out", [128, cfg.NT * cfg.F2],
                                          DT.float32, isOutput=True)

    with tile.TileContext(nc) as tc:
        with tc.tile_pool(name="const", bufs=1) as cpool, \
             tc.tile_pool(name="idx", bufs=cfg.GBUFS) as idxpool, \
             tc.tile_pool(name="dl", bufs=cfg.GBUFS) as dlpool, \
             tc.tile_pool(name="g0", bufs=cfg.GBUFS) as g0, \
             tc.tile_pool(name="g1", bufs=cfg.GBUFS) as g1, \
             tc.tile_pool(name="g2", bufs=cfg.GBUFS) as g2, \
             tc.tile_pool(name="g3", bufs=cfg.GBUFS) as g3, \
             tc.tile_pool(name="own", bufs=2) as ownpool, \
             tc.tile_pool(name="S", bufs=2) as spool, \
             tc.tile_pool(name="work", bufs=3) as wpool, \
             tc.tile_pool(name="pacc", bufs=2, space="PSUM") as pacc, \
             tc.tile_pool(name="pmisc", bufs=2, space="PSUM") as pmisc:
            gpools = [g0, g1, g2, g3][:cfg.SEG]
            dinv = _dinv_tiles(nc, cpool, cnt_in, cfg)
            bvec = cpool.tile([128, FIN], DT.float32, tag="bvec")
            nc.scalar.dma_start(out=bvec[:], in_=b_in[:])
            ident = cpool.tile([128, 128], gdt, tag="ident")
            nc.scalar.dma_start(out=ident[:], in_=ident_in[:])
            iota = cpool.tile([128, cfg.TG * 128], DT.float32, tag="iota")
            nc.scalar.dma_start(out=iota[:], in_=iota_in[:])
            if RUNTIME_COUNTS:
                cnts_sb = cpool.tile([128, cfg.NG * cfg.SEG], DT.int32,
                                     tag="cnts")
                nc.scalar.dma_start(out=cnts_sb[:], in_=cnts_in[:])
            if layer == 1:
                w2b = cpool.tile([128, cfg.F2], DT.bfloat16, tag="w2b")
                nc.scalar.dma_start(out=w2b[:], in_=w2_in[:])
                identb = ident  # bf16 already
            for g in range(cfg.NG):
                tiles = cfg.groups[g]
                ntg = len(tiles)
                t0 = tiles[0]
                go, cg = int(meta.goff[g]), meta.capg[g]

                idxg = idxpool.tile([128, cg // 16], DT.int16, tag="idxg")
                nc.sync.dma_start(out=idxg[:],
                                  in_=idx_in[:, go // 16:(go + cg) // 16])
                dlg = dlpool.tile([128, cg // 128], DT.float32, tag="dlg")
                nc.sync.dma_start(out=dlg[:],
                                  in_=dl_in[:, go // 128:(go + cg) // 128])

                Gt = {}
                for s in range(cfg.SEG):
                    csz = int(meta.cap[g, s])
                    if csz == 0:
                        continue
                    so = int(meta.soff[g, s])
                    if RUNTIME_COUNTS:
                        reg = nc.gpsimd.value_load(
                            cnts_sb[0:1, g * cfg.SEG + s:g * cfg.SEG + s + 1],
                            min_val=0, max_val=csz)
                    else:
                        reg = csz
                    Gs = gpools[s].tile([128, csz // 128, FROW], gdt,
                                        tag=f"G{s}")
                    nc.gpsimd.dma_gather(
                        out_ap=Gs[:],
                        in_ap=tab_in[s][:, :],
                        idxs_ap=idxg[:, so // 16:(so + csz) // 16],
                        num_idxs=csz,
                        num_idxs_reg=reg,
                        elem_size=FROW,
                        single_packet=SINGLE_PACKET,
                        queue_num=s,
                    )
                    Gt[s] = Gs

                ownw = ownpool.tile([128, ntg, FROW], gdt, tag="own")
                nc.scalar.dma_start(
                    out=ownw[:],
                    in_=own_in[:, t0 * FROW:(t0 + ntg) * FROW]
                    .rearrange("p (a f) -> p a f", f=FROW))

                acc_g = pacc.tile([128, ntg, FIN], DT.float32, tag="acc")
                for j, t in enumerate(tiles):
                    rng = meta.ranges.get((g, j), [])
                    nbt = sum(b1 - b0 for _, b0, b1 in rng)
                    # self term first (start), aggregation blocks after
                    nc.tensor.matmul(acc_g[:, j, :], ident[:],
                                     ownw[:, j, 0:FIN],
                                     start=True, stop=(nbt == 0))
                    if nbt == 0:
                        continue
                    St = spool.tile([128, nbt, 128], sdt, tag="St")
                    ci = 0
                    for s, b0, b1 in rng:
                        nb = b1 - b0
                        nc.vector.tensor_tensor(
                            St[:, ci:ci + nb, :],
                            iota[:, j * 128:(j + 1) * 128]
                            .unsqueeze(1).broadcast_to((128, nb, 128)),
                            dlg[:, b0:b1].unsqueeze(2)
                            .broadcast_to((128, nb, 128)),
                            op=ALU.is_equal)
                        ci += nb
                    ci = 0
                    for s, b0, b1 in rng:
                        base = int(meta.soff[g, s]) // 128
                        for k in range(b0, b1):
                            nc.tensor.matmul(
                                acc_g[:, j, :], St[:, ci, :],
                                Gt[s][:, k - base, 0:FIN],
                                start=False,
                                stop=(ci == nbt - 1))
                            ci += 1

                dinv_bc = dinv[:, t0:t0 + ntg].unsqueeze(2).broadcast_to(
                    (128, ntg, FIN))
                bvec_bc = bvec[:].unsqueeze(1).broadcast_to((128, ntg, FIN))
                zw = wpool.tile([128, ntg, FIN], DT.float32, tag="zw")
                nc.vector.tensor_tensor(zw[:], acc_g[:], dinv_bc, op=ALU.mult)
                nc.vector.tensor_tensor(zw[:], zw[:], bvec_bc, op=ALU.add)

                if layer == 1:
                    h1b = wpool.tile([128, ntg, cfg.F1], DT.bfloat16,
                                     tag="h1b")
                    nc.scalar.activation(h1b[:], zw[:], ACTF.Relu)
                    t2p = pmisc.tile([128, ntg, cfg.F2], DT.float32,
                                     tag="t2p")
                    for j in range(ntg):
                        hTp = pmisc.tile([128, 128], DT.bfloat16, tag="hTp")
                        nc.tensor.transpose(hTp[:], h1b[:, j, :], identb[:])
                        hTb = wpool.tile([128, 128], DT.bfloat16, tag="hTb")
                        nc.scalar.activation(hTb[:], hTp[:], ACTF.Copy)
                        nc.tensor.matmul(t2p[:, j, :], hTb[:], w2b[:],
                                         start=True, stop=True)
                    dinv_bc2 = dinv[:, t0:t0 + ntg].unsqueeze(2).broadcast_to(
                        (128, ntg, cfg.F2))
                    ht2w = wpool.tile([128, ntg, cfg.F2], DT.float32,
                                      tag="ht2w")
                    nc.vector.tensor_tensor(ht2w[:], t2p[:], dinv_bc2,
                                            op=ALU.mult)
                    # 256B table rows: 64 features duplicated, bf16
                    htbw = wpool.tile([128, ntg, 2, cfg.F2], DT.bfloat16,
                                      tag="htbw")
                    nc.vector.tensor_copy(
                        htbw[:],
                        ht2w[:].unsqueeze(2).broadcast_to(
                            (128, ntg, 2, cfg.F2)))
                    nc.scalar.dma_start(
                        out=out_t[:, t0 * FROW:(t0 + ntg) * FROW],
                        in_=htbw[:].rearrange("p a two f -> p (a two f)"))
                else:
                    mxw = wpool.tile([128, ntg], DT.float32, tag="mxw")
                    nc.vector.tensor_reduce(mxw[:], zw[:],
                                            axis=mybir.AxisListType.X,
                                            op=ALU.max, negate=True)
                    ssumw = wpool.tile([128, ntg], DT.float32, tag="ssumw")
                    for j in range(ntg):
                        e = wpool.tile([128, cfg.F2], DT.float32, tag="e")
                        nc.scalar.activation(e[:], zw[:, j, :], ACTF.Exp,
                                             bias=mxw[:, j:j + 1],
                                             accum_out=ssumw[:, j:j + 1])
                    lsew = wpool.tile([128, ntg], DT.float32, tag="lsew")
                    nc.scalar.activation(lsew[:], ssumw[:], ACTF.Ln)
                    nc.vector.tensor_tensor(
                        zw[:], zw[:],
                        mxw[:].unsqueeze(2).broadcast_to((128, ntg, cfg.F2)),
                        op=ALU.add)
                    nc.vector.tensor_tensor(
                        zw[:], zw[:],
                        lsew[:].unsqueeze(2).broadcast_to((128, ntg, cfg.F2)),
                        op=ALU.subtract)
                    nc.scalar.dma_start(
                        out=out_t[:, t0 * cfg.F2:(t0 + ntg) * cfg.F2],
                        in_=zw[:].rearrange("p a f -> p (a f)"))
    nc.compile()
    return nc


# ----------------------------------------------------------------------------
# Runner
# ----------------------------------------------------------------------------


def _install_ntff_hook():
    try:
        import antenv
        if "antenv.axon_hooks" not in sys.modules:
            from trn_agent_boot.trn_boot import _ntff_profile_via_ctypes
            hooks = types.ModuleType("antenv.axon_hooks")
            holder = {"hook": _ntff_profile_via_ctypes("/opt/axon/libaxon_pjrt.so")}
            hooks.get_axon_ntff_profile_hook = lambda: holder["hook"]
            hooks.set_axon_ntff_profile_hook = lambda h: holder.__setitem__("hook", h)
            sys.modules["antenv.axon_hooks"] = hooks
            antenv.axon_hooks = hooks
    except Exception:
        pass


_CACHE = {}
LAST_EXEC_NS = []


def _get_programs(cfg, meta, key):
    if key not in _CACHE:
        _CACHE[key] = (build_transform1(cfg),
                       build_agg(cfg, meta, 1),
                       build_agg(cfg, meta, 2))
    return _CACHE[key]


def _marshal_xT(cfg, x):
    """x [N, F0] f32 -> per-core [128, NCH, KB, CT*128] bf16 (k-major)."""
    out = np.zeros((cfg.NCORES, 128, cfg.NCH, cfg.KB, cfg.CT * 128), BF16)
    for c in range(cfg.NCORES):
        xs = x[c * cfg.NPC:(c + 1) * cfg.NPC]          # [NPC, F0]
        xp = np.zeros((cfg.NTP, cfg.F0), np.float32)
        xp[:cfg.NPC] = xs
        # [NT*128, KB, 128] -> [128(k), NCH, KB, CT*128(node)]
        xk = xp.reshape(cfg.NTP, cfg.KB, 128)
        xk = xk.transpose(2, 1, 0).reshape(128, cfg.KB, cfg.NCH, cfg.CT * 128)
        out[c] = xk.transpose(0, 2, 1, 3).astype(BF16)
    return out


def kernel(x, edge_index, W1, b1, W2, b2):
    cfg = Cfg()
    x = np.asarray(x, dtype=np.float32)
    edge_index = np.asarray(edge_index)
    W1 = np.asarray(W1, dtype=np.float32)
    b1 = np.asarray(b1, dtype=np.float32)
    W2 = np.asarray(W2, dtype=np.float32)
    b2 = np.asarray(b2, dtype=np.float32)

    trace = os.environ.get("GCN_TRACE", "0") == "1"
    if trace:
        _install_ntff_hook()

    meta = preprocess(cfg, edge_index)
    key = hash(edge_index.tobytes())
    p1, p2, p3 = _get_programs(cfg, meta, key)

    xT = _marshal_xT(cfg, x)
    w1b = np.ascontiguousarray(
        W1.reshape(cfg.KB, 128, cfg.F1).transpose(1, 0, 2)).astype(BF16)
    w2b = W2.astype(BF16)
    identb_v = np.eye(128, dtype=np.float32).astype(BF16)
    iota_v = np.broadcast_to(
        np.arange(cfg.TG * 128, dtype=np.float32), (128, cfg.TG * 128)).copy()
    b1b = np.broadcast_to(b1, (128, cfg.F1)).copy()
    b2b = np.broadcast_to(b2, (128, cfg.F2)).copy()
    cores = list(range(cfg.NCORES))

    global LAST_EXEC_NS
    LAST_EXEC_NS = []

    # Launch 1: transform
    maps1 = [{"xT": xT[c], "w1b": w1b, "cnt": meta.cnt_dev[c]}
             for c in cores]
    r1 = run_bass_kernel_spmd(p1, maps1, cores, trace=trace)
    LAST_EXEC_NS.append(r1.exec_time_ns)
    ht1b = np.stack([r1.results[c]["ht1b"] for c in cores], axis=0)
    # global table: row c*NTP + p*NT + t <- ht1b[c][p, t*F1:+F1]
    tabg1 = ht1b.reshape(cfg.NCORES * 128 * cfg.NT, cfg.F1)
    segs1 = {f"tab{si}": np.ascontiguousarray(
        tabg1[si * cfg.SEGSZ:(si + 1) * cfg.SEGSZ])
        for si in range(cfg.SEG)}

    # Launch 2: layer-1 aggregation + transform-2
    maps2 = [{**segs1, "own": ht1b[c], "cnt": meta.cnt_dev[c],
              "idx": meta.idx_dev[c], "dl": meta.dl_dev[c],
              **({"cnts": meta.cnts_dev[c]} if RUNTIME_COUNTS else {}),
              "iota": iota_v, "bvec": b1b,
              "ident": identb_v, "w2b": w2b} for c in cores]
    r2 = run_bass_kernel_spmd(p2, maps2, cores, trace=trace)
    LAST_EXEC_NS.append(r2.exec_time_ns)
    ht2b = np.stack([r2.results[c]["ht2b"] for c in cores], axis=0)
    tabg2 = ht2b.reshape(cfg.NCORES * 128 * cfg.NT, 128)
    segs2 = {f"tab{si}": np.ascontiguousarray(
        tabg2[si * cfg.SEGSZ:(si + 1) * cfg.SEGSZ])
        for si in range(cfg.SEG)}

    # Launch 3: layer-2 aggregation + log_softmax
    maps3 = [{**segs2, "own": ht2b[c], "cnt": meta.cnt_dev[c],
              "idx": meta.idx_dev[c], "dl": meta.dl_dev[c],
              **({"cnts": meta.cnts_dev[c]} if RUNTIME_COUNTS else {}),
              "iota": iota_v, "bvec": b2b,
              "ident": identb_v} for c in cores]
    r3 = run_bass_kernel_spmd(p3, maps3, cores, trace=trace)
    LAST_EXEC_NS.append(r3.exec_time_ns)

    out = np.empty((cfg.N, cfg.F2), np.float32)
    for c in cores:
        o = r3.results[c]["out"].reshape(128, cfg.NT, cfg.F2)
        out[c * cfg.NPC:(c + 1) * cfg.NPC] = (
            o.transpose(1, 0, 2).reshape(cfg.NTP, cfg.F2)[:cfg.NPC])
    return out


# revision 26
# speedup vs baseline: 1.3873x; 1.1211x over previous
"""Distributed 2-layer GCN (PyG GCNConv semantics) on 8 Trainium2 NeuronCores.

Strategy: nodes sharded across 8 cores (12500 each); edges bucketed by
(dst-core, dst-tile-group, src-segment) via host-side 1D partitioning.
Three SPMD launches:

  1. transform1:  ht1 = dinv * (x @ W1)   (host-pretransposed bf16 x -> no
     PE transposes; single partition-major bf16 output table)
  2. agg1+xform2: gather ht1b[src] rows per edge (4 SWDGE queues), segmented
     one-hot matmul scatter-add per dst tile, self-loop added in PSUM via an
     identity matmul, relu, ht2 = dinv*(h1 @ W2); f32 partition-major out.
  3. agg2+logsoftmax over f32 ht2 table (64-feat rows = 256B gathers).

Perf notes vs the earlier version (2.15ms -> target < 1.1ms):
  - slot padding trimmed 25%->~3%: edges packed contiguously per
    (group, seg) bucket with *group-level* dst encodings (dl in [0,TG*128)),
    so matmul block boundaries no longer have to align with dst tiles.
    Boundary blocks simply run one matmul per touched tile; the group-level
    dl encoding makes the one-hot rows of other tiles zero automatically.
  - per-core runtime descriptor counts: num_idxs_reg is value_load'ed from
    a per-core counts table, so DMA descriptors track the real per-core
    edge count while the instruction stream stays identical across cores.
    (The first G-pool-depth groups gather their full capacity to flush
    stale SBUF so untrimmed pad blocks never feed NaNs to the PE.)
  - all regular DMA is big-packet: tables/own/outputs use partition-major
    [128, NT*F] layouts (the host undoes the permutation for free).
  - L2 self term rides the PSUM accumulator (identity-stationary matmul)
    instead of two DVE passes.
"""

import os
import sys
import types

for _p in ("/opt/trn_rl_repo", "/root/.axon_site/_ro/trn_rl_repo", "/root/.axon_site"):
    if os.path.isdir(_p) and _p not in sys.path:
        sys.path.insert(0, _p)

import numpy as np
import ml_dtypes

from concourse import bass, bacc, tile
from concourse.bass_utils import run_bass_kernel_spmd

mybir = bass.mybir
DT = bass.mybir.dt
ALU = mybir.AluOpType
ACTF = mybir.ActivationFunctionType
BF16 = ml_dtypes.bfloat16

# Runtime (register-sourced) gather counts are unusable in this runtime:
# reg_load from SBUF/DRAM halts the engine (NRT_EXEC_UNIT_UNRECOVERABLE).
RUNTIME_COUNTS = os.environ.get("GCN_RUNTIME_COUNTS", "0") == "1"
SINGLE_PACKET = os.environ.get("GCN_SINGLE_PACKET", "0") == "1"

# ----------------------------------------------------------------------------
# Configuration
# ----------------------------------------------------------------------------


class Cfg:
    def __init__(self, N=100000, E=1600000, F0=256, F1=128, F2=64,
                 NCORES=8, SEG=4, TG=4, GBUFS=4):
        self.N = N
        self.E = E
        self.F0 = F0
        self.F1 = F1
        self.F2 = F2
        self.NCORES = NCORES
        self.NPC = N // NCORES            # nodes per core
        self.NT = -(-self.NPC // 128)     # dst tiles per core
        self.LAST_ROWS = self.NPC - (self.NT - 1) * 128
        self.NTP = self.NT * 128          # padded rows per core
        self.NGL = NCORES * self.NTP      # global padded table rows
        self.SEG = SEG
        assert self.NGL % SEG == 0
        self.SEGSZ = self.NGL // SEG
        assert self.SEGSZ <= 32767
        self.TG = TG                      # dst tiles per gather group
        self.NG = -(-self.NT // TG)
        self.groups = [list(range(g * TG, min((g + 1) * TG, self.NT)))
                       for g in range(self.NG)]
        self.GBUFS = GBUFS                # gather pool depth == warm groups
        self.KB = F0 // 128               # k blocks for transform 1
        # transform-1 input chunking (tiles per chunk) for DMA/compute overlap
        self.CT = 14 if self.NT % 14 == 0 else self.NT
        self.NCH = self.NT // self.CT


class Meta:
    """Edge partitioning metadata; identical across cores (static program)."""
    pass


def preprocess(cfg, edge_index):
    """1D graph partitioning of the edge list. Pure integer index work."""
    src = np.asarray(edge_index[0], dtype=np.int64)
    dst = np.asarray(edge_index[1], dtype=np.int64)

    cnt = np.bincount(dst, minlength=cfg.N).astype(np.int64)

    core = dst // cfg.NPC
    within = dst % cfg.NPC
    tile_id = within // 128
    dloc = within % 128
    g_id = tile_id // cfg.TG
    t_in_g = tile_id % cfg.TG
    gdl = t_in_g * 128 + dloc             # group-level dst encoding

    # node permutation: node (c, t, p) -> table row c*NTP + p*NT + t
    sc = src // cfg.NPC
    sw = src % cfg.NPC
    st = sw // 128
    sp = sw % 128
    prow = sc * cfg.NTP + sp * cfg.NT + st
    seg = prow // cfg.SEGSZ
    sloc = prow % cfg.SEGSZ

    # bucket (core, group, seg); within bucket sort by (tile, sloc)
    bucket = (core * cfg.NG + g_id) * cfg.SEG + seg
    skey = (bucket * cfg.TG + t_in_g) * np.int64(cfg.SEGSZ) + sloc
    order = np.argsort(skey, kind="stable")
    sloc_s = sloc[order].astype(np.int16)
    gdl_s = gdl[order].astype(np.float32)
    tig_s = t_in_g[order].astype(np.int64)
    bucket_s = bucket[order]

    nbuckets = cfg.NCORES * cfg.NG * cfg.SEG
    bc = np.bincount(bucket, minlength=nbuckets).reshape(
        cfg.NCORES, cfg.NG, cfg.SEG)
    bstart = np.zeros(nbuckets + 1, np.int64)
    np.cumsum(bc.reshape(-1), out=bstart[1:])
    # per (core, g, s, t) counts for block ranges
    bct = np.zeros((cfg.NCORES, cfg.NG, cfg.SEG, cfg.TG), np.int64)
    np.add.at(bct, (core, g_id, seg, t_in_g), 1)

    m = Meta()
    if RUNTIME_COUNTS:
        # uniform capacities so gather pool buffers keep one shape: warm
        # groups initialize every byte, runtime-trimmed gathers never expose
        # uninitialized SBUF to the PE.
        capv = -(-bc.max(axis=(0, 1)) // 128) * 128             # [SEG]
        cap = np.broadcast_to(capv, (cfg.NG, cfg.SEG)).astype(np.int64).copy()
    else:
        # static counts: every slot is gathered (pads idx=0), so caps can be
        # per-bucket minima = ceil(max-over-cores / 128).
        cap = (-(-bc.max(axis=0) // 128) * 128).astype(np.int64)
    m.cap = cap
    m.soff = np.zeros((cfg.NG, cfg.SEG), np.int64)  # slot offset within group
    m.goff = np.zeros(cfg.NG + 1, np.int64)          # group slot offset, global
    for g in range(cfg.NG):
        off = 0
        for s in range(cfg.SEG):
            m.soff[g, s] = off
            off += int(cap[g, s])
        m.goff[g + 1] = m.goff[g] + off
    m.capg = [int(m.goff[g + 1] - m.goff[g]) for g in range(cfg.NG)]
    m.tot = int(m.goff[cfg.NG])

    # per-core slot arrays. Pad slots: dl=-1 always (one-hot rows stay zero);
    # idx=-1 for runtime-trimmed groups (negative tail = not gathered, must
    # match the count register exactly), idx=0 for warm full-capacity groups
    # (gathered harmlessly so every pool-buffer byte gets initialized).
    idx_all = np.full((cfg.NCORES, m.tot), -1, np.int16)
    dl_all = np.full((cfg.NCORES, m.tot), -1.0, np.float32)
    for c in range(cfg.NCORES):
        for g in range(cfg.NG):
            for s in range(cfg.SEG):
                b = (c * cfg.NG + g) * cfg.SEG + s
                k = int(bc[c, g, s])
                o = int(m.goff[g]) + int(m.soff[g, s])
                if k:
                    e0 = int(bstart[b])
                    idx_all[c, o:o + k] = sloc_s[e0:e0 + k]
                    dl_all[c, o:o + k] = gdl_s[e0:e0 + k]
                if not RUNTIME_COUNTS or g < cfg.GBUFS:
                    idx_all[c, o + k:o + int(cap[g, s])] = 0

    # block ranges per (g, t): union over cores, per segment; block index is
    # group-relative (matches the per-group dl tile).
    m.ranges = {}
    for g in range(cfg.NG):
        for j, t in enumerate(cfg.groups[g]):
            lst = []
            for s in range(cfg.SEG):
                lo, hi = None, None
                for c in range(cfg.NCORES):
                    k = int(bct[c, g, s, j])
                    if k == 0:
                        continue
                    st_ = int(bct[c, g, s, :j].sum())
                    b0 = (int(m.soff[g, s]) + st_) // 128
                    b1 = -(-(int(m.soff[g, s]) + st_ + k) // 128)
                    lo = b0 if lo is None else min(lo, b0)
                    hi = b1 if hi is None else max(hi, b1)
                if lo is not None:
                    lst.append((s, lo, hi))
            m.ranges[(g, j)] = lst

    # device layouts: per-group 16-wrap idx and 128-wrap dl, concatenated
    idx_dev = np.zeros((cfg.NCORES, 128, m.tot // 16), np.int16)
    dl_dev = np.zeros((cfg.NCORES, 128, m.tot // 128), np.float32)
    for g in range(cfg.NG):
        o, cg = int(m.goff[g]), m.capg[g]
        i16 = idx_all[:, o:o + cg].reshape(cfg.NCORES, cg // 16, 16)
        idx_dev[:, :, o // 16:(o + cg) // 16] = np.tile(
            i16.transpose(0, 2, 1), (1, 8, 1))
        d128 = dl_all[:, o:o + cg].reshape(cfg.NCORES, cg // 128, 128)
        dl_dev[:, :, o // 128:(o + cg) // 128] = d128.transpose(0, 2, 1)
    m.idx_dev = np.ascontiguousarray(idx_dev)
    m.dl_dev = np.ascontiguousarray(dl_dev)

    # runtime gather counts: real per-core count, except warm groups (full)
    cnts = np.zeros((cfg.NCORES, cfg.NG * cfg.SEG), np.int32)
    for c in range(cfg.NCORES):
        for g in range(cfg.NG):
            for s in range(cfg.SEG):
                k = int(bc[c, g, s])
                cnts[c, g * cfg.SEG + s] = int(cap[g, s]) if g < cfg.GBUFS else k
    m.cnts_dev = np.ascontiguousarray(
        np.broadcast_to(cnts[:, None, :], (cfg.NCORES, 128, cfg.NG * cfg.SEG)))

    # degree counts per core as f32 [128, NT] (node t*128+p <-> [p, t])
    pad = cfg.NTP - cfg.NPC
    cnt_dev = np.zeros((cfg.NCORES, 128, cfg.NT), np.float32)
    for c in range(cfg.NCORES):
        cc = cnt[c * cfg.NPC:(c + 1) * cfg.NPC]
        cc = np.concatenate([cc, np.zeros(pad, np.int64)])
        cnt_dev[c] = cc.reshape(cfg.NT, 128).T.astype(np.float32)
    m.cnt_dev = cnt_dev
    return m


# ----------------------------------------------------------------------------
# Program builders
# ----------------------------------------------------------------------------


def _dinv_tiles(nc, pool, cnt_in, cfg):
    """dinv = 1/sqrt(cnt + 1) as an SBUF [128, NT] f32 tile."""
    cnt_sb = pool.tile([128, cfg.NT], DT.float32, tag="cnt")
    nc.scalar.dma_start(out=cnt_sb[:], in_=cnt_in[:])
    deg = pool.tile([128, cfg.NT], DT.float32, tag="deg")
    nc.vector.tensor_scalar_add(deg[:], cnt_sb[:], 1.0)
    sq = pool.tile([128, cfg.NT], DT.float32, tag="sq")
    nc.scalar.sqrt(sq[:], deg[:])
    dinv = pool.tile([128, cfg.NT], DT.float32, tag="dinv")
    nc.vector.reciprocal(dinv[:], sq[:])
    return dinv


def build_transform1(cfg):
    """ht1b = bf16(dinv * (x @ W1)), partition-major [128, NT*F1] output.

    x arrives host-transposed/bf16 as [128, NCH, KB, CT*128] (k-partition,
    chunk, k-block, node-col); W1 bf16 [128, KB, F1]."""
    nc = bacc.Bacc(None, target_bir_lowering=False)
    xT_in = nc.declare_dram_parameter(
        "xT", [128, cfg.NCH, cfg.KB, cfg.CT * 128], DT.bfloat16, isOutput=False)
    w1_in = nc.declare_dram_parameter("w1b", [128, cfg.KB, cfg.F1],
                                      DT.bfloat16, isOutput=False)
    cnt_in = nc.declare_dram_parameter("cnt", [128, cfg.NT], DT.float32,
                                       isOutput=False)
    out_t = nc.declare_dram_parameter("ht1b", [128, cfg.NT * cfg.F1],
                                      DT.bfloat16, isOutput=True)

    with tile.TileContext(nc) as tc:
        with tc.tile_pool(name="const", bufs=1) as cpool, \
             tc.tile_pool(name="xin", bufs=2) as xpool, \
             tc.tile_pool(name="psum", bufs=4, space="PSUM") as ppool:
            dinv = _dinv_tiles(nc, cpool, cnt_in, cfg)
            w1sb = cpool.tile([128, cfg.KB, cfg.F1], DT.bfloat16, tag="w1")
            nc.sync.dma_start(out=w1sb[:], in_=w1_in[:])
            outsb = cpool.tile([128, cfg.NT * cfg.F1], DT.bfloat16, tag="out")
            # pad partitions of the last tile are never computed; zero them so
            # the table/own rows they become stay finite.
            nc.vector.memset(outsb[:], 0.0)

            CW = cfg.CT * cfg.F1
            for ch in range(cfg.NCH):
                xt = xpool.tile([128, cfg.KB, cfg.CT * 128], DT.bfloat16,
                                tag="xt")
                nc.sync.dma_start(out=xt[:], in_=xT_in[:, ch, :, :])
                for lt in range(cfg.CT):
                    t = ch * cfg.CT + lt
                    rows = cfg.LAST_ROWS if t == cfg.NT - 1 else 128
                    hp = ppool.tile([128, cfg.F1], DT.float32, tag="hp")
                    for kb in range(cfg.KB):
                        nc.tensor.matmul(
                            hp[:rows, :],
                            xt[:, kb, lt * 128:lt * 128 + rows],
                            w1sb[:, kb, :],
                            start=(kb == 0), stop=(kb == cfg.KB - 1))
                    nc.vector.tensor_scalar_mul(
                        outsb[:rows, t * cfg.F1:(t + 1) * cfg.F1],
                        hp[:rows, :], dinv[:rows, t:t + 1])
                nc.scalar.dma_start(out=out_t[:, ch * CW:(ch + 1) * CW],
                                    in_=outsb[:, ch * CW:(ch + 1) * CW])
    nc.compile()
    return nc


def build_agg(cfg, meta, layer):
    """layer 1: aggregate ht1b -> h1 -> ht2 = dinv*(h1 @ W2); f32 out.
       layer 2: aggregate ht2 -> +b2 -> log_softmax; f32 out.

    Per group: SEG gathers (one per SWDGE queue) with runtime per-core
    descriptor counts; per dst tile a one-hot stationary built from the
    group-level dl encoding; self term accumulated into PSUM via an
    identity matmul; wide DVE eviction."""
    # FIN = aggregated feature width; FROW = table row width (256B bf16 rows:
    # layer-2 rows hold the 64 features duplicated). fp32 tables/matmuls are a
    # trap: fp32 stationary loads don't pipeline and the PE becomes the wall.
    FIN = cfg.F1 if layer == 1 else cfg.F2
    FROW = 128
    gdt = sdt = DT.bfloat16
    nc = bacc.Bacc(None, target_bir_lowering=False, num_swdge_queues=4)
    tab_in = [nc.declare_dram_parameter(f"tab{si}", [cfg.SEGSZ, FROW], gdt,
                                        isOutput=False)
              for si in range(cfg.SEG)]
    own_in = nc.declare_dram_parameter("own", [128, cfg.NT * FROW], gdt,
                                       isOutput=False)
    cnt_in = nc.declare_dram_parameter("cnt", [128, cfg.NT], DT.float32,
                                       isOutput=False)
    idx_in = nc.declare_dram_parameter("idx", [128, meta.tot // 16], DT.int16,
                                       isOutput=False)
    dl_in = nc.declare_dram_parameter("dl", [128, meta.tot // 128], DT.float32,
                                      isOutput=False)
    cnts_in = None
    if RUNTIME_COUNTS:
        cnts_in = nc.declare_dram_parameter("cnts", [128, cfg.NG * cfg.SEG],
                                            DT.int32, isOutput=False)
    iota_in = nc.declare_dram_parameter("iota", [128, cfg.TG * 128],
                                        DT.float32, isOutput=False)
    b_in = nc.declare_dram_parameter("bvec", [128, FIN], DT.float32,
                                     isOutput=False)
    ident_in = nc.declare_dram_parameter("ident", [128, 128], gdt,
                                         isOutput=False)
    if layer == 1:
        w2_in = nc.declare_dram_parameter("w2b", [128, cfg.F2], DT.bfloat16,
                                          isOutput=False)
        out_t = nc.declare_dram_parameter("ht2b", [128, cfg.NT * FROW],
                                          DT.bfloat16, isOutput=True)
    else:
        out_t = nc.declare_dram_parameter("out", [128, cfg.NT * cfg.F2],
                                          DT.float32, isOutput=True)

    with tile.TileContext(nc) as tc:
        with tc.tile_pool(name="const", bufs=1) as cpool, \
             tc.tile_pool(name="idx", bufs=cfg.GBUFS) as idxpool, \
             tc.tile_pool(name="dl", bufs=cfg.GBUFS) as dlpool, \
             tc.tile_pool(name="g0", bufs=cfg.GBUFS) as g0, \
             tc.tile_pool(name="g1", bufs=cfg.GBUFS) as g1, \
             tc.tile_pool(name="g2", bufs=cfg.GBUFS) as g2, \
             tc.tile_pool(name="g3", bufs=cfg.GBUFS) as g3, \
             tc.tile_pool(name="own", bufs=2) as ownpool, \
             tc.tile_pool(name="S", bufs=2) as spool, \
             tc.tile_pool(name="work", bufs=3) as wpool, \
             tc.tile_pool(name="pacc", bufs=2, space="PSUM") as pacc, \
             tc.tile_pool(name="pmisc", bufs=2, space="PSUM") as pmisc:
            gpools = [g0, g1, g2, g3][:cfg.SEG]
            dinv = _dinv_tiles(nc, cpool, cnt_in, cfg)
            bvec = cpool.tile([128, FIN], DT.float32, tag="bvec")
            nc.scalar.dma_start(out=bvec[:], in_=b_in[:])
            ident = cpool.tile([128, 128], gdt, tag="ident")
            nc.scalar.dma_start(out=ident[:], in_=ident_in[:])
            iota = cpool.tile([128, cfg.TG * 128], DT.float32, tag="iota")
            nc.scalar.dma_start(out=iota[:], in_=iota_in[:])
            if RUNTIME_COUNTS:
                cnts_sb = cpool.tile([128, cfg.NG * cfg.SEG], DT.int32,
                                     tag="cnts")
                nc.scalar.dma_start(out=cnts_sb[:], in_=cnts_in[:])
            if layer == 1:
                w2b = cpool.tile([128, cfg.F2], DT.bfloat16, tag="w2b")
                nc.scalar.dma_start(out=w2b[:], in_=w2_in[:])
                identb = ident  # bf16 already
            for g in range(cfg.NG):
                tiles = cfg.groups[g]
                ntg = len(tiles)
                t0 = tiles[0]
                go, cg = int(meta.goff[g]), meta.capg[g]

                idxg = idxpool.tile([128, cg // 16], DT.int16, tag="idxg")
                nc.sync.dma_start(out=idxg[:],
                                  in_=idx_in[:, go // 16:(go + cg) // 16])
                dlg = dlpool.tile([128, cg // 128], DT.float32, tag="dlg")
                nc.sync.dma_start(out=dlg[:],
                                  in_=dl_in[:, go // 128:(go + cg) // 128])

                Gt = {}
                for s in range(cfg.SEG):
                    csz = int(meta.cap[g, s])
                    if csz == 0:
                        continue
                    so = int(meta.soff[g, s])
                    if RUNTIME_COUNTS:
                        reg = nc.gpsimd.value_load(
                            cnts_sb[0:1, g * cfg.SEG + s:g * cfg.SEG + s + 1],
                            min_val=0, max_val=csz)
                    else:
                        reg = csz
                    Gs = gpools[s].tile([128, csz // 128, FROW], gdt,
                                        tag=f"G{s}")
                    nc.gpsimd.dma_gather(
                        out_ap=Gs[:],
                        in_ap=tab_in[s][:, :],
                        idxs_ap=idxg[:, so // 16:(so + csz) // 16],
                        num_idxs=csz,
                        num_idxs_reg=reg,
                        elem_size=FROW,
                        single_packet=SINGLE_PACKET,
                        queue_num=s,
                    )
                    Gt[s] = Gs

                ownw = ownpool.tile([128, ntg, FROW], gdt, tag="own")
                nc.scalar.dma_start(
                    out=ownw[:],
                    in_=own_in[:, t0 * FROW:(t0 + ntg) * FROW]
                    .rearrange("p (a f) -> p a f", f=FROW))

                acc_g = pacc.tile([128, ntg, FIN], DT.float32, tag="acc")
                for j, t in enumerate(tiles):
                    rng = meta.ranges.get((g, j), [])
                    nbt = sum(b1 - b0 for _, b0, b1 in rng)
                    # self term first (start), aggregation blocks after
                    nc.tensor.matmul(acc_g[:, j, :], ident[:],
                                     ownw[:, j, 0:FIN],
                                     start=True, stop=(nbt == 0))
                    if nbt == 0:
                        continue
                    St = spool.tile([128, nbt, 128], sdt, tag="St")
                    ci = 0
                    for s, b0, b1 in rng:
                        nb = b1 - b0
                        nc.vector.tensor_tensor(
                            St[:, ci:ci + nb, :],
                            iota[:, j * 128:(j + 1) * 128]
                            .unsqueeze(1).broadcast_to((128, nb, 128)),
                            dlg[:, b0:b1].unsqueeze(2)
                            .broadcast_to((128, nb, 128)),
                            op=ALU.is_equal)
                        ci += nb
                    ci = 0
                    for s, b0, b1 in rng:
                        base = int(meta.soff[g, s]) // 128
                        for k in range(b0, b1):
                            nc.tensor.matmul(
                                acc_g[:, j, :], St[:, ci, :],
                                Gt[s][:, k - base, 0:FIN],
                                start=False,
                                stop=(ci == nbt - 1))
                            ci += 1

                dinv_bc = dinv[:, t0:t0 + ntg].unsqueeze(2).broadcast_to(
                    (128, ntg, FIN))
                bvec_bc = bvec[:].unsqueeze(1).broadcast_to((128, ntg, FIN))
                zw = wpool.tile([128, ntg, FIN], DT.float32, tag="zw")
                nc.vector.tensor_tensor(zw[:], acc_g[:], dinv_bc, op=ALU.mult)
                nc.vector.tensor_tensor(zw[:], zw[:], bvec_bc, op=ALU.add)

                if layer == 1:
                    h1b = wpool.tile([128, ntg, cfg.F1], DT.bfloat16,
                                     tag="h1b")
                    nc.scalar.activation(h1b[:], zw[:], ACTF.Relu)
                    t2p = pmisc.tile([128, ntg, cfg.F2], DT.float32,
                                     tag="t2p")
                    for j in range(ntg):
                        hTp = pmisc.tile([128, 128], DT.bfloat16, tag="hTp")
                        nc.tensor.transpose(hTp[:], h1b[:, j, :], identb[:])
                        hTb = wpool.tile([128, 128], DT.bfloat16, tag="hTb")
                        nc.scalar.activation(hTb[:], hTp[:], ACTF.Copy)
                        nc.tensor.matmul(t2p[:, j, :], hTb[:], w2b[:],
                                         start=True, stop=True)
                    dinv_bc2 = dinv[:, t0:t0 + ntg].unsqueeze(2).broadcast_to(
                        (128, ntg, cfg.F2))
                    ht2w = wpool.tile([128, ntg, cfg.F2], DT.float32,
                                      tag="ht2w")
                    nc.vector.tensor_tensor(ht2w[:], t2p[:], dinv_bc2,
                                            op=ALU.mult)
                    # 256B table rows: 64 features duplicated, bf16
                    htbw = wpool.tile([128, ntg, 2, cfg.F2], DT.bfloat16,
                                      tag="htbw")
                    nc.vector.tensor_copy(
                        htbw[:],
                        ht2w[:].unsqueeze(2).broadcast_to(
                            (128, ntg, 2, cfg.F2)))
                    nc.scalar.dma_start(
                        out=out_t[:, t0 * FROW:(t0 + ntg) * FROW],
                        in_=htbw[:].rearrange("p a two f -> p (a two f)"))
                else:
                    mxw = wpool.tile([128, ntg], DT.float32, tag="mxw")
                    nc.vector.tensor_reduce(mxw[:], zw[:],
                                            axis=mybir.AxisListType.X,
                                            op=ALU.max, negate=True)
                    ssumw = wpool.tile([128, ntg], DT.float32, tag="ssumw")
                    for j in range(ntg):
                        e = wpool.tile([128, cfg.F2], DT.float32, tag="e")
                        nc.scalar.activation(e[:], zw[:, j, :], ACTF.Exp,
                                             bias=mxw[:, j:j + 1],
                                             accum_out=ssumw[:, j:j + 1])
                    lsew = wpool.tile([128, ntg], DT.float32, tag="lsew")
                    nc.scalar.activation(lsew[:], ssumw[:], ACTF.Ln)
                    nc.vector.tensor_tensor(
                        zw[:], zw[:],
                        mxw[:].unsqueeze(2).broadcast_to((128, ntg, cfg.F2)),
                        op=ALU.add)
                    nc.vector.tensor_tensor(
                        zw[:], zw[:],
                        lsew[:].unsqueeze(2).broadcast_to((128, ntg, cfg.F2)),
                        op=ALU.subtract)
                    nc.scalar.dma_start(
                        out=out_t[:, t0 * cfg.F2:(t0 + ntg) * cfg.F2],
                        in_=zw[:].rearrange("p a f -> p (a f)"))
    nc.compile()
    return nc


# ----------------------------------------------------------------------------
# Runner
# ----------------------------------------------------------------------------


def _install_ntff_hook():
    try:
        import antenv
        if "antenv.axon_hooks" not in sys.modules:
            from trn_agent_boot.trn_boot import _ntff_profile_via_ctypes
            hooks = types.ModuleType("antenv.axon_hooks")
            holder = {"hook": _ntff_profile_via_ctypes("/opt/axon/libaxon_pjrt.so")}
            hooks.get_axon_ntff_profile_hook = lambda: holder["hook"]
            hooks.set_axon_ntff_profile_hook = lambda h: holder.__setitem__("hook", h)
            sys.modules["antenv.axon_hooks"] = hooks
            antenv.axon_hooks = hooks
    except Exception:
        pass


_CACHE = {}
LAST_EXEC_NS = []


def _get_programs(cfg, meta, key):
    if key not in _CACHE:
        _CACHE[key] = (build_transform1(cfg),
                       build_agg(cfg, meta, 1),
                       build_agg(cfg, meta, 2))
    return _CACHE[key]


def _marshal_xT(cfg, x):
    """x [N, F0] f32 -> per-core [128, NCH, KB, CT*128] bf16 (k-major)."""
    out = np.zeros((cfg.NCORES, 128, cfg.NCH, cfg.KB, cfg.CT * 128), BF16)
    for c in range(cfg.NCORES):
        xs = x[c * cfg.NPC:(c + 1) * cfg.NPC]          # [NPC, F0]
        xp = np.zeros((cfg.NTP, cfg.F0), np.float32)
        xp[:cfg.NPC] = xs
        # [NT*128, KB, 128] -> [128(k), NCH, KB, CT*128(node)]
        xk = xp.reshape(cfg.NTP, cfg.KB, 128)
        xk = xk.transpose(2, 1, 0).reshape(128, cfg.KB, cfg.NCH, cfg.CT * 128)
        out[c] = xk.transpose(0, 2, 1, 3).astype(BF16)
    return out


def kernel(x, edge_index, W1, b1, W2, b2):
    cfg = Cfg()
    x = np.asarray(x, dtype=np.float32)
    edge_index = np.asarray(edge_index)
    W1 = np.asarray(W1, dtype=np.float32)
    b1 = np.asarray(b1, dtype=np.float32)
    W2 = np.asarray(W2, dtype=np.float32)
    b2 = np.asarray(b2, dtype=np.float32)

    trace = os.environ.get("GCN_TRACE", "0") == "1"
    if trace:
        _install_ntff_hook()

    meta = preprocess(cfg, edge_index)
    key = hash(edge_index.tobytes())
    p1, p2, p3 = _get_programs(cfg, meta, key)

    xT = _marshal_xT(cfg, x)
    w1b = np.ascontiguousarray(
        W1.reshape(cfg.KB, 128, cfg.F1).transpose(1, 0, 2)).astype(BF16)
    w2b = W2.astype(BF16)
    identb_v = np.eye(128, dtype=np.float32).astype(BF16)
    iota_v = np.broadcast_to(
        np.arange(cfg.TG * 128, dtype=np.float32), (128, cfg.TG * 128)).copy()
    b1b = np.broadcast_to(b1, (128, cfg.F1)).copy()
    b2b = np.broadcast_to(b2, (128, cfg.F2)).copy()
    cores = list(range(cfg.NCORES))

    global LAST_EXEC_NS
    LAST_EXEC_NS = []

    # Launch 1: transform
    maps1 = [{"xT": xT[c], "w1b": w1b, "cnt": meta.cnt_dev[c]}
             for c in cores]
    r1 = run_bass_kernel_spmd(p1, maps1, cores, trace=trace)
    LAST_EXEC_NS.append(r1.exec_time_ns)
    ht1b = np.stack([r1.results[c]["ht1b"] for c in cores], axis=0)
    # global table: row c*NTP + p*NT + t <- ht1b[c][p, t*F1:+F1]
    tabg1 = ht1b.reshape(cfg.NCORES * 128 * cfg.NT, cfg.F1)
    segs1 = {f"tab{si}": np.ascontiguousarray(
        tabg1[si * cfg.SEGSZ:(si + 1) * cfg.SEGSZ])
        for si in range(cfg.SEG)}

    # Launch 2: layer-1 aggregation + transform-2
    maps2 = [{**segs1, "own": ht1b[c], "cnt": meta.cnt_dev[c],
              "idx": meta.idx_dev[c], "dl": meta.dl_dev[c],
              **({"cnts": meta.cnts_dev[c]} if RUNTIME_COUNTS else {}),
              "iota": iota_v, "bvec": b1b,
              "ident": identb_v, "w2b": w2b} for c in cores]
    r2 = run_bass_kernel_spmd(p2, maps2, cores, trace=trace)
    LAST_EXEC_NS.append(r2.exec_time_ns)
    ht2b = np.stack([r2.results[c]["ht2b"] for c in cores], axis=0)
    tabg2 = ht2b.reshape(cfg.NCORES * 128 * cfg.NT, 128)
    segs2 = {f"tab{si}": np.ascontiguousarray(
        tabg2[si * cfg.SEGSZ:(si + 1) * cfg.SEGSZ])
        for si in range(cfg.SEG)}

    # Launch 3: layer-2 aggregation + log_softmax
    maps3 = [{**segs2, "own": ht2b[c], "cnt": meta.cnt_dev[c],
              "idx": meta.idx_dev[c], "dl": meta.dl_dev[c],
              **({"cnts": meta.cnts_dev[c]} if RUNTIME_COUNTS else {}),
              "iota": iota_v, "bvec": b2b,
              "ident": identb_v} for c in cores]
    r3 = run_bass_kernel_spmd(p3, maps3, cores, trace=trace)
    LAST_EXEC_NS.append(r3.exec_time_ns)

    out = np.empty((cfg.N, cfg.F2), np.float32)
    for c in cores:
        o = r3.results[c]["out"].reshape(128, cfg.NT, cfg.F2)
        out[c * cfg.NPC:(c + 1) * cfg.NPC] = (
            o.transpose(1, 0, 2).reshape(cfg.NTP, cfg.F2)[:cfg.NPC])
    return out
